# revision 30
# baseline (speedup 1.0000x reference)
"""Trainium2 Bass kernel for nn_Decoder_Model_EBV (gnn_message_passing).

Math: score[e] = <X_trans[src_e] - X_trans[tgt_e], ebvecs[type_e]>
      with X_trans = X_embed @ W.T.

Folding W into the basis vectors: U = ebvecs @ W  (500 x 512), and
Z = X_embed @ U.T  (100000 x 500) gives
      score[e] = Z[src_e, type_e] - Z[tgt_e, type_e].

Sharding: nodes are split evenly across the 8 NeuronCores (12500 each).
The host supplies X pre-transposed in fp16 as xt[kc, p, n] =
X[n, kc*128+p] so the device runs a single clean stream of fp16
matmuls (1 PE cycle/row, no on-device transposes):

  warmup:   dependency-free matmuls ramp the PE clock to 2.4 GHz
  prologue: UT[e, t] = sum_b W[b, e] * ebvecs.T[b, t]   (8 small matmuls)
  main:     ZT[t, n] = sum_e UT[e, t] * XT[e, n]        (400 matmuls,
            4 type-blocks x 4 K-chunks x 25 chunks of 500 nodes,
            500 rows each, back-to-back at ~211 ns => PE-bound at the
            78.6 TF/s fp16 roofline)

PSUM tiles are evacuated as fp16 alternately on the DVE and Act engines,
batched in pairs, and DMA'd on the Sync/GpSimd queues (inputs stream on
Sync with a ramp-up group schedule so the first tiles arrive early).
ZT is written out as fp16 g[tb, p, n] = Z[n, tb*128+p]; the host picks
score[e] = Z[src, t] - Z[tgt, t] from the owning cores' tables
(vertex-cut, zero cross-device communication).

Precision: fp16 x/U with fp32 PSUM accumulation gives max-err/absmax
4.4e-4 (50x under the 2e-2 gate). fp8 e4m3 (the only faster PE mode,
2x via DoubleRow) was measured at 2.2e-2..2.5e-2 on the real data for
every half-K split — over the gate — and residual-correction schemes
cost back the speedup, so fp16 is the optimum here.
"""

import numpy as np

import concourse.bass as bass
import concourse.bacc as bacc
import concourse.tile as tile
import concourse.mybir as mybir
from concourse.bass_utils import run_bass_kernel_spmd

# problem constants (hardcoded per spec)
N_NODES = 100000
EMBED = 512
BASIS = 256
NREL = 500
E = 300000

NCORES = 8
NPC = N_NODES // NCORES          # 12500 nodes per core
P = 128
NT = 500                         # moving-dim tile (25 per core, 1 PSUM bank)
NSUB = NPC // NT                 # 25 node tiles
GROUPS = [1, 2, 4, 9, 9]         # node tiles per input DMA group (ramp-up)
GMAX = max(GROUPS)

_compiled = None


def _build_program():
    nc = bacc.Bacc("TRN2", target_bir_lowering=False, debug=False,
                   num_devices=NCORES)
    f32 = mybir.dt.float32
    f16 = mybir.dt.float16

    xt_ap = nc.dram_tensor("xt", [4, P, NPC], f16, kind="ExternalInput").ap()
    # packed [w0 w1 eb0 eb1], each [128, 512] fp16 (ebt host-padded with
    # zeros past the 500 real type columns)
    weh_ap = nc.dram_tensor("weh", [P, 4 * EMBED], f16,
                            kind="ExternalInput").ap()
    g_ap = nc.dram_tensor("g", [4, P, NPC], f16, kind="ExternalOutput").ap()

    with tile.TileContext(nc) as tc:
        with tc.tile_pool(name="const", bufs=1) as cpool, \
             tc.tile_pool(name="xin", bufs=3) as xpool, \
             tc.tile_pool(name="zo", bufs=6) as opool, \
             tc.tile_pool(name="fold_ps", bufs=2, space="PSUM") as fpool, \
             tc.tile_pool(name="z_ps", bufs=6, space="PSUM") as zpool:

            # ---- prologue: UT = (ebvecs @ W).T in fp16 ----
            # ebt is host-padded to 512 type columns (zeros past NREL) so
            # the fold directly produces the full zero-padded ut table.
            # split across all three DMA queues so the fold's inputs land
            # as early as possible (early-kernel DMA rate is well below peak)
            # (sync is kept free for the xg input stream so its first
            # descriptors issue as early as possible)
            weh = cpool.tile([P, 4 * EMBED], f16, tag="weh")
            nc.scalar.dma_start(out=weh[:, :EMBED], in_=weh_ap[:, :EMBED])
            nc.gpsimd.dma_start(out=weh[:, EMBED:2 * EMBED],
                                in_=weh_ap[:, EMBED:2 * EMBED])
            nc.gpsimd.dma_start(out=weh[:, 2 * EMBED:3 * EMBED],
                                in_=weh_ap[:, 2 * EMBED:3 * EMBED])
            nc.scalar.dma_start(out=weh[:, 3 * EMBED:],
                                in_=weh_ap[:, 3 * EMBED:])

            # PE warm-up during the input DMA: dependency-free matmuls ramp
            # the tensor-engine clock to full speed before the fold runs.
            # (warmup psums use the main-loop pool, not the fold pool — a
            # shared tag would make the first fold matmul WAW-wait on a
            # warmup tile, and that idle gap resets the PE clock ramp)
            wz = cpool.tile([P, EMBED], f16, tag="wz")
            nc.vector.memset(wz[:], 0.0)
            for r in range(11):
                wps = zpool.tile([P, NT], f32, tag="zp")
                nc.tensor.matmul(out=wps[:], lhsT=wz[:, :P],
                                 rhs=wz[:, :NT], start=True, stop=True)

            # ut layout: ut[:, kc*512 + t] = UT[kc*128+p, t]
            ut = cpool.tile([P, 4 * EMBED], f16, tag="ut")
            for mb in range(4):
                up = fpool.tile([P, EMBED], f32, tag="up")
                for c in range(2):
                    nc.tensor.matmul(
                        out=up[:],
                        lhsT=weh[:, c * EMBED + mb * P:
                                 c * EMBED + (mb + 1) * P],
                        rhs=weh[:, (2 + c) * EMBED:(3 + c) * EMBED],
                        start=(c == 0), stop=(c == 1))
                if mb % 2 == 0:
                    nc.vector.tensor_copy(
                        out=ut[:, mb * EMBED:(mb + 1) * EMBED], in_=up[:])
                else:
                    nc.scalar.copy(
                        out=ut[:, mb * EMBED:(mb + 1) * EMBED], in_=up[:])

            # ---- main: stream node tiles, 16 matmuls each ----
            starts = [sum(GROUPS[:i]) for i in range(len(GROUPS))]

            def load_group(gi):
                ns = GROUPS[gi] * NT
                base = starts[gi] * NT
                xg = xpool.tile([P, 4 * GMAX * NT], f16, tag="xg")
                for kc in range(4):
                    nc.sync.dma_start(
                        out=xg[:, kc * GMAX * NT:kc * GMAX * NT + ns],
                        in_=xt_ap[kc][:, base:base + ns])
                return xg

            xg = load_group(0)
            zbs = None
            ndma = 0
            for gi in range(len(GROUPS)):
                xg_cur = xg
                if gi + 1 < len(GROUPS):
                    xg = load_group(gi + 1)
                for sub in range(GROUPS[gi]):
                    gsub = starts[gi] + sub
                    half = gsub % 2
                    if half == 0:
                        zbs = [opool.tile([P, 2 * NT], f16, tag=f"zb{tb}",
                                          name=f"zb{tb}")
                               for tb in range(4)]
                    for tb in range(4):
                        zp = zpool.tile([P, NT], f32, tag="zp")
                        for kc in range(4):
                            nc.tensor.matmul(
                                out=zp[:],
                                lhsT=ut[:, kc * EMBED + tb * P:
                                        kc * EMBED + (tb + 1) * P],
                                rhs=xg_cur[:, kc * GMAX * NT + sub * NT:
                                           kc * GMAX * NT + (sub + 1) * NT],
                                start=(kc == 0), stop=(kc == 3))
                        if tb % 2 == 0:
                            nc.vector.tensor_copy(
                                out=zbs[tb][:, half * NT:(half + 1) * NT],
                                in_=zp[:])
                        else:
                            nc.scalar.copy(
                                out=zbs[tb][:, half * NT:(half + 1) * NT],
                                in_=zp[:])
                        if half == 1 or gsub == NSUB - 1:
                            w = (half + 1) * NT
                            n0 = (gsub - half) * NT
                            # inputs are done by the tail; let the idle sync
                            # queue take the last outputs so gpsimd drains
                            if gsub >= NSUB - 6:
                                eng = nc.sync
                            else:
                                eng = nc.sync if ndma % 2 == 0 else nc.gpsimd
                            eng.dma_start(out=g_ap[tb][:, n0:n0 + w],
                                          in_=zbs[tb][:, :w])
                            ndma += 1

    nc.compile()
    return nc


def _prep_inputs(X_embed, edge_list_pred, edge_type_pred, W, ebvecs):
    """Shard inputs across cores; build per-core pick index tables."""
    X_embed = np.asarray(X_embed, dtype=np.float32)
    W = np.asarray(W, dtype=np.float32)
    ebvecs = np.asarray(ebvecs, dtype=np.float32)

    weh = np.zeros((P, 4 * EMBED), dtype=np.float16)
    weh[:, :EMBED] = W[:P].astype(np.float16)
    weh[:, EMBED:2 * EMBED] = W[P:].astype(np.float16)
    ebt16 = ebvecs.T.astype(np.float16)            # [256, 500]
    weh[:, 2 * EMBED:2 * EMBED + NREL] = ebt16[:P]
    weh[:, 3 * EMBED:3 * EMBED + NREL] = ebt16[P:]
    xt_all = X_embed.T.astype(np.float16)          # [512, 100000]

    src = np.asarray(edge_list_pred[0], dtype=np.int64)
    tgt = np.asarray(edge_list_pred[1], dtype=np.int64)
    ty = np.asarray(edge_type_pred).reshape(-1).astype(np.int64)

    nodes = np.concatenate([src, tgt])                 # 600000
    types = np.concatenate([ty, ty])
    edges = np.concatenate([np.arange(E), np.arange(E)])
    signs = np.concatenate([np.ones(E, np.float32), -np.ones(E, np.float32)])

    owner = nodes // NPC                               # 0..7
    nloc = nodes - owner * NPC
    tb = types // P
    tp = types % P

    in_maps = []
    pick = []  # per core: (tb, partition, node_col, edges, signs)
    for i in range(NCORES):
        xt = np.ascontiguousarray(
            xt_all[:, i * NPC:(i + 1) * NPC].reshape(4, P, NPC))
        in_maps.append({"xt": xt, "weh": weh})
        sel = owner == i
        pick.append((tb[sel], tp[sel], nloc[sel], edges[sel], signs[sel]))
    return in_maps, pick


def kernel(X_embed, edge_list_pred, edge_type_pred, W, ebvecs,
           _trace=False, _tmpdir=None):
    global _compiled
    if _compiled is None:
        _compiled = _build_program()
    nc = _compiled

    in_maps, pick = _prep_inputs(X_embed, edge_list_pred, edge_type_pred,
                                 W, ebvecs)
    kw = {}
    if _trace:
        kw = {"trace": True, "tmpdir": _tmpdir}
    res = run_bass_kernel_spmd(nc, in_maps, list(range(NCORES)), **kw)

    scores = np.zeros(E, dtype=np.float64)
    for i in range(NCORES):
        tbs, tps, cols, ed, sg = pick[i]
        vals = res.results[i]["g"][tbs, tps, cols].astype(np.float64)
        scores += np.bincount(ed, weights=sg * vals, minlength=E)
    out = scores.astype(np.float32).reshape(1, E)
    if _trace:
        kernel.last_exec_time_ns = res.exec_time_ns
        kernel.last_results = res
    return out


# revision 32
# speedup vs baseline: 1.0395x; 1.0395x over previous
"""Trainium2 Bass kernel for nn_Decoder_Model_EBV (gnn_message_passing).

Math: score[e] = <X_trans[src_e] - X_trans[tgt_e], ebvecs[type_e]>
      with X_trans = X_embed @ W.T.

Folding W into the basis vectors: U = ebvecs @ W  (500 x 512), and
Z = X_embed @ U.T  (100000 x 500) gives
      score[e] = Z[src_e, type_e] - Z[tgt_e, type_e].

Sharding: nodes are split evenly across the 8 NeuronCores (12500 each).
The host supplies X pre-transposed in fp16 as xt[kc, p, n] =
X[n, kc*128+p] so the device runs a single clean stream of fp16
matmuls (1 PE cycle/row, no on-device transposes):

  warmup:   dependency-free matmuls ramp the PE clock to 2.4 GHz
  prologue: UT[e, t] = sum_b W[b, e] * ebvecs.T[b, t]   (8 small matmuls)
  main:     ZT[t, n] = sum_e UT[e, t] * XT[e, n]        (400 matmuls,
            4 type-blocks x 4 K-chunks x 25 chunks of 500 nodes,
            500 rows each, back-to-back at ~211 ns => PE-bound at the
            78.6 TF/s fp16 roofline)

PSUM tiles are evacuated as fp16 alternately on the DVE and Act engines,
batched in pairs, and DMA'd on the Sync/GpSimd queues (inputs stream on
Sync with a ramp-up group schedule so the first tiles arrive early).
ZT is written out as fp16 g[tb, p, n] = Z[n, tb*128+p]; the host picks
score[e] = Z[src, t] - Z[tgt, t] from the owning cores' tables
(vertex-cut, zero cross-device communication).

Precision: fp16 x/U with fp32 PSUM accumulation gives max-err/absmax
4.4e-4 (50x under the 2e-2 gate). fp8 e4m3 (the only faster PE mode,
2x via DoubleRow) was measured at 2.2e-2..2.5e-2 on the real data for
every half-K split — over the gate — and residual-correction schemes
cost back the speedup, so fp16 is the optimum here.
"""

import numpy as np

import concourse.bass as bass
import concourse.bacc as bacc
import concourse.tile as tile
import concourse.mybir as mybir
from concourse.bass_utils import run_bass_kernel_spmd

# problem constants (hardcoded per spec)
N_NODES = 100000
EMBED = 512
BASIS = 256
NREL = 500
E = 300000

NCORES = 8
NPC = N_NODES // NCORES          # 12500 nodes per core
P = 128
NT = 500                         # moving-dim tile (25 per core, 1 PSUM bank)
NSUB = NPC // NT                 # 25 node tiles
GROUPS = [1, 2, 4, 6, 6, 5, 1]   # node tiles per input DMA group (ramp-up)
GMAX = max(GROUPS)

_compiled = None


def _build_program():
    nc = bacc.Bacc("TRN2", target_bir_lowering=False, debug=False,
                   num_devices=NCORES)
    f32 = mybir.dt.float32
    f16 = mybir.dt.float16

    xt_ap = nc.dram_tensor("xt", [4, P, NPC], f16, kind="ExternalInput").ap()
    # packed [w0 w1 eb0 eb1], each [128, 512] fp16 (ebt host-padded with
    # zeros past the 500 real type columns)
    weh_ap = nc.dram_tensor("weh", [P, 4 * EMBED], f16,
                            kind="ExternalInput").ap()
    g_ap = nc.dram_tensor("g", [4, P, NPC], f16, kind="ExternalOutput").ap()

    with tile.TileContext(nc) as tc:
        with tc.tile_pool(name="const", bufs=1) as cpool, \
             tc.tile_pool(name="xin", bufs=3) as xpool, \
             tc.tile_pool(name="zo", bufs=6) as opool, \
             tc.tile_pool(name="fold_ps", bufs=2, space="PSUM") as fpool, \
             tc.tile_pool(name="z_ps", bufs=6, space="PSUM") as zpool:

            # ---- prologue: UT = (ebvecs @ W).T in fp16 ----
            # ebt is host-padded to 512 type columns (zeros past NREL) so
            # the fold directly produces the full zero-padded ut table.
            # split across all three DMA queues so the fold's inputs land
            # as early as possible (early-kernel DMA rate is well below peak)
            # (sync is kept free for the xg input stream so its first
            # descriptors issue as early as possible)
            weh = cpool.tile([P, 4 * EMBED], f16, tag="weh")
            nc.scalar.dma_start(out=weh[:, :EMBED], in_=weh_ap[:, :EMBED])
            nc.gpsimd.dma_start(out=weh[:, EMBED:2 * EMBED],
                                in_=weh_ap[:, EMBED:2 * EMBED])
            nc.gpsimd.dma_start(out=weh[:, 2 * EMBED:3 * EMBED],
                                in_=weh_ap[:, 2 * EMBED:3 * EMBED])
            nc.scalar.dma_start(out=weh[:, 3 * EMBED:],
                                in_=weh_ap[:, 3 * EMBED:])

            # PE warm-up during the input DMA: dependency-free matmuls ramp
            # the tensor-engine clock to full speed before the fold runs.
            # (warmup psums use the main-loop pool, not the fold pool — a
            # shared tag would make the first fold matmul WAW-wait on a
            # warmup tile, and that idle gap resets the PE clock ramp)
            wz = cpool.tile([P, EMBED], f16, tag="wz")
            nc.vector.memset(wz[:], 0.0)
            for r in range(11):
                wps = zpool.tile([P, NT], f32, tag="zp")
                nc.tensor.matmul(out=wps[:], lhsT=wz[:, :P],
                                 rhs=wz[:, :NT], start=True, stop=True)

            # ut layout: ut[:, kc*512 + t] = UT[kc*128+p, t]
            ut = cpool.tile([P, 4 * EMBED], f16, tag="ut")
            for mb in range(4):
                up = fpool.tile([P, EMBED], f32, tag="up")
                for c in range(2):
                    nc.tensor.matmul(
                        out=up[:],
                        lhsT=weh[:, c * EMBED + mb * P:
                                 c * EMBED + (mb + 1) * P],
                        rhs=weh[:, (2 + c) * EMBED:(3 + c) * EMBED],
                        start=(c == 0), stop=(c == 1))
                if mb % 2 == 0:
                    nc.vector.tensor_copy(
                        out=ut[:, mb * EMBED:(mb + 1) * EMBED], in_=up[:])
                else:
                    nc.scalar.copy(
                        out=ut[:, mb * EMBED:(mb + 1) * EMBED], in_=up[:])

            # ---- main: stream node tiles, 16 matmuls each ----
            starts = [sum(GROUPS[:i]) for i in range(len(GROUPS))]

            def load_group(gi):
                ns = GROUPS[gi] * NT
                base = starts[gi] * NT
                xg = xpool.tile([P, 4 * GMAX * NT], f16, tag="xg")
                for kc in range(4):
                    nc.sync.dma_start(
                        out=xg[:, kc * GMAX * NT:kc * GMAX * NT + ns],
                        in_=xt_ap[kc][:, base:base + ns])
                return xg

            # prefetch two groups ahead (3 xpool bufs) so every group's DMA
            # has a full extra group-time of margin — a late tile stalls the
            # PE and resets its clock ramp, costing ~3x the gap itself
            pending = [load_group(0), load_group(1)]
            zbs = None
            ndma = 0
            for gi in range(len(GROUPS)):
                xg_cur = pending.pop(0)
                if gi + 2 < len(GROUPS):
                    pending.append(load_group(gi + 2))
                for sub in range(GROUPS[gi]):
                    gsub = starts[gi] + sub
                    half = gsub % 2
                    if half == 0:
                        zbs = [opool.tile([P, 2 * NT], f16, tag=f"zb{tb}",
                                          name=f"zb{tb}")
                               for tb in range(4)]
                    for tb in range(4):
                        zp = zpool.tile([P, NT], f32, tag="zp")
                        for kc in range(4):
                            nc.tensor.matmul(
                                out=zp[:],
                                lhsT=ut[:, kc * EMBED + tb * P:
                                        kc * EMBED + (tb + 1) * P],
                                rhs=xg_cur[:, kc * GMAX * NT + sub * NT:
                                           kc * GMAX * NT + (sub + 1) * NT],
                                start=(kc == 0), stop=(kc == 3))
                        if tb % 2 == 0:
                            nc.vector.tensor_copy(
                                out=zbs[tb][:, half * NT:(half + 1) * NT],
                                in_=zp[:])
                        else:
                            nc.scalar.copy(
                                out=zbs[tb][:, half * NT:(half + 1) * NT],
                                in_=zp[:])
                        if half == 1 or gsub == NSUB - 1:
                            w = (half + 1) * NT
                            n0 = (gsub - half) * NT
                            # inputs are done by the tail; let the idle sync
                            # queue take the last outputs so gpsimd drains
                            if gsub >= NSUB - 6:
                                eng = nc.sync
                            else:
                                eng = nc.sync if ndma % 2 == 0 else nc.gpsimd
                            eng.dma_start(out=g_ap[tb][:, n0:n0 + w],
                                          in_=zbs[tb][:, :w])
                            ndma += 1

    nc.compile()
    return nc


def _prep_inputs(X_embed, edge_list_pred, edge_type_pred, W, ebvecs):
    """Shard inputs across cores; build per-core pick index tables."""
    X_embed = np.asarray(X_embed, dtype=np.float32)
    W = np.asarray(W, dtype=np.float32)
    ebvecs = np.asarray(ebvecs, dtype=np.float32)

    weh = np.zeros((P, 4 * EMBED), dtype=np.float16)
    weh[:, :EMBED] = W[:P].astype(np.float16)
    weh[:, EMBED:2 * EMBED] = W[P:].astype(np.float16)
    ebt16 = ebvecs.T.astype(np.float16)            # [256, 500]
    weh[:, 2 * EMBED:2 * EMBED + NREL] = ebt16[:P]
    weh[:, 3 * EMBED:3 * EMBED + NREL] = ebt16[P:]
    xt_all = X_embed.T.astype(np.float16)          # [512, 100000]

    src = np.asarray(edge_list_pred[0], dtype=np.int64)
    tgt = np.asarray(edge_list_pred[1], dtype=np.int64)
    ty = np.asarray(edge_type_pred).reshape(-1).astype(np.int64)

    nodes = np.concatenate([src, tgt])                 # 600000
    types = np.concatenate([ty, ty])
    edges = np.concatenate([np.arange(E), np.arange(E)])
    signs = np.concatenate([np.ones(E, np.float32), -np.ones(E, np.float32)])

    owner = nodes // NPC                               # 0..7
    nloc = nodes - owner * NPC
    tb = types // P
    tp = types % P

    in_maps = []
    pick = []  # per core: (tb, partition, node_col, edges, signs)
    for i in range(NCORES):
        xt = np.ascontiguousarray(
            xt_all[:, i * NPC:(i + 1) * NPC].reshape(4, P, NPC))
        in_maps.append({"xt": xt, "weh": weh})
        sel = owner == i
        pick.append((tb[sel], tp[sel], nloc[sel], edges[sel], signs[sel]))
    return in_maps, pick


def kernel(X_embed, edge_list_pred, edge_type_pred, W, ebvecs,
           _trace=False, _tmpdir=None):
    global _compiled
    if _compiled is None:
        _compiled = _build_program()
    nc = _compiled

    in_maps, pick = _prep_inputs(X_embed, edge_list_pred, edge_type_pred,
                                 W, ebvecs)
    kw = {}
    if _trace:
        kw = {"trace": True, "tmpdir": _tmpdir}
    res = run_bass_kernel_spmd(nc, in_maps, list(range(NCORES)), **kw)

    scores = np.zeros(E, dtype=np.float64)
    for i in range(NCORES):
        tbs, tps, cols, ed, sg = pick[i]
        vals = res.results[i]["g"][tbs, tps, cols].astype(np.float64)
        scores += np.bincount(ed, weights=sg * vals, minlength=E)
    out = scores.astype(np.float32).reshape(1, E)
    if _trace:
        kernel.last_exec_time_ns = res.exec_time_ns
        kernel.last_results = res
    return out
